# revision 1
# baseline (speedup 1.0000x reference)
"""ClassicalSelfAttention TRN2 kernel — 8-core SPMD, sequence-parallel, fp32r.

out = softmax((X Wq)(X Wk)^T / sqrt(d)) @ X,  X:[4096,1024] f32, W:[1024,1024].

Per core (rows sharded 8x512), all matmuls single-pass fp32r (1 cyc/row at
free-dim >= 256), PV in fp16:
  Q^T[f,m] = sum_e Wq[e,f] XlT[e,m]          (lhsT=wq as-is, rhs=xlT)
  B^T[b,m] = sum_f WkT[f,b] Q^T[f,m]         (lhsT=wkT, rhs=QT)
  S[m,j]   = sum_b BT[b,m] XT[b,j]           (stream XT, 2MB chunks)
  P        = softmax(S/32): one big exp per m-tile, accum_out = rowsum
  PT[j,m]  = PE-transpose of P chunks (fp16)
  out      = (PT^T-contracted with streamed fp16 X rows) * 1/rowsum,
             split into two m-pair passes so the first PV pass overlaps the
             second half of softmax (PSUM: 4 transpose banks + 4 PV banks)

Host pre-transposes X^T / Wk^T and pre-casts X to fp16. fp32r logit error
measured on HW: rms ~3.4 abs on |S|~3e4 (tf32-ish) vs top-2 logit gaps
almost always >> that; PV fp16 adds ~5e-4 rel. Long matmul runs between
semaphore waits keep the PE clock-gate warm (2.4GHz needs ~3us of backlog);
dummy identity-transposes pad the few unavoidable seams (P1->S, S->softmax,
PV-B start) so the tensor engine never de-ramps.
"""
import numpy as np
import concourse.bass as bass
import concourse.bacc as bacc
import concourse.mybir as mybir
import concourse.tile as tile
from concourse import masks
from concourse.bass_utils import run_bass_kernel_spmd

F32 = mybir.dt.float32
F32R = mybir.dt.float32r
F16 = mybir.dt.float16

D = 1024          # embed dim
NT = 4096         # tokens
NC = 8            # cores
NL = NT // NC     # 512 local rows
DT = D // 128     # 8 d-tiles
MT = NL // 128    # 4 m-tiles
JC = NT // 512    # 8 j-chunks (S free-dim blocks of 512)
C4 = NT // 1024   # 4 streamed xT chunks
JT = NT // 128    # 32 j-tiles
SCALE = float(1.0 / np.sqrt(np.float32(D)))

EXP = mybir.ActivationFunctionType.Exp
COPY = mybir.ActivationFunctionType.Copy


def build_nc():
    nc = bacc.Bacc("TRN2", target_bir_lowering=False, debug=False)

    xf16 = nc.declare_dram_parameter("xf16", [NT, D], F16, isOutput=False)
    xt_full = nc.declare_dram_parameter("xt_full", [D, NT], F32, isOutput=False)
    xlt = nc.declare_dram_parameter("xlt", [D, NL], F32, isOutput=False)
    wq = nc.declare_dram_parameter("wq", [D, D], F32, isOutput=False)
    wkt = nc.declare_dram_parameter("wkt", [D, D], F32, isOutput=False)
    out_l = nc.declare_dram_parameter("out_local", [NL, D], F32, isOutput=True)

    with tile.TileContext(nc) as tc:
        with (
            tc.tile_pool(name="persist", bufs=1) as persist,
            tc.tile_pool(name="stats", bufs=1) as stats,
        ):
            ident = persist.tile([128, 128], F32, tag="ident", name="ident")
            masks.make_identity(nc, ident[:])
            BT = [persist.tile([128, NL], F32R, tag=f"BT{b}", name=f"BT{b}")
                  for b in range(DT)]
            pmax = [stats.tile([128, JC], F32, tag=f"pmax{m}", name=f"pmax{m}") for m in range(MT)]
            esum = [stats.tile([128, 2], F32, tag=f"esum{m}", name=f"esum{m}") for m in range(MT)]
            rsum = [stats.tile([128, 1], F32, tag=f"rsum{m}", name=f"rsum{m}") for m in range(MT)]
            recip = stats.tile([128, MT], F32, tag="recip", name="recip")

            # ---------- P1: Q^T then B^T (small projection algebra) ----------
            with (
                tc.tile_pool(name="p1", bufs=1) as p1,
                tc.tile_pool(name="ps1", bufs=1, space=bass.MemorySpace.PSUM) as ps1,
            ):
                wq_t = p1.tile([128, DT, D], F32R, tag="wq", name="wq")
                xl_t = p1.tile([128, DT, NL], F32R, tag="xl", name="xl")
                wk_t = p1.tile([128, DT, D], F32R, tag="wk", name="wk")
                qt_t = [p1.tile([128, NL], F32R, tag=f"qt{f}", name=f"qt{f}") for f in range(DT)]
                # fine-grained per-chunk loads: keeps the PE fed from the first
                # chunk on (coarse loads make the PE SEQ block long enough to
                # drain the exec queue and de-ramp the clock)
                nc.sync.dma_start(xl_t[:, 0, :], xlt[0:128, :].bitcast(F32R))
                nc.sync.dma_start(wq_t[:, 0, 0:512], wq[0:128, 0:512].bitcast(F32R))
                nc.sync.dma_start(wq_t[:, 0, 512:1024], wq[0:128, 512:1024].bitcast(F32R))
                for e in range(1, DT):
                    nc.sync.dma_start(
                        wq_t[:, e, :], wq[e * 128:(e + 1) * 128, :].bitcast(F32R))
                    nc.sync.dma_start(
                        xl_t[:, e, :], xlt[e * 128:(e + 1) * 128, :].bitcast(F32R))
                for f in range(DT):
                    nc.sync.dma_start(
                        wk_t[:, f, :], wkt[f * 128:(f + 1) * 128, :].bitcast(F32R))

                # Q^T: e-outer so each arriving (wq, xlT) chunk is consumed at once
                pq = [ps1.tile([128, NL], F32, tag=f"pq{f}", name=f"pq{f}") for f in range(DT)]
                for e in range(DT):
                    for f in range(DT):
                        nc.tensor.matmul(pq[f][:], wq_t[:, e, f * 128:(f + 1) * 128],
                                         xl_t[:, e, :], start=(e == 0), stop=(e == DT - 1))
                for f in range(DT):
                    nc.vector.tensor_copy(qt_t[f][:], pq[f][:])

                pb = [ps1.tile([128, NL], F32, tag=f"pq{b}", name=f"pb{b}") for b in range(DT)]
                for f in range(DT):
                    for b in range(DT):
                        nc.tensor.matmul(pb[b][:], wk_t[:, f, b * 128:(b + 1) * 128],
                                         qt_t[f][:], start=(f == 0), stop=(f == DT - 1))
                for b in range(DT):
                    nc.vector.tensor_copy(BT[b][:], pb[b][:])
                # keep the PE engine busy across the P1->S seam: idle gaps
                # de-ramp the clock-gate and cost ~20us of slow matmuls
                for w in range(30):
                    nc.tensor.transpose(pb[0][:, 0:128], ident[:], ident[:])

            # ---------- persistent P^T + exp scratch ----------
            with (
                tc.tile_pool(name="ptb", bufs=1) as ptb,
                tc.tile_pool(name="pch", bufs=2) as pch,
            ):
                # P^T, one plane per m-tile: [128 j-part, jt, 128 m-cols], fp16
                PTm = [ptb.tile([128, JT, 128], F16, tag=f"PT{m}", name=f"PT{m}")
                       for m in range(MT)]

                with tc.tile_pool(name="sbig", bufs=1) as sbig:
                    S = [sbig.tile([128, JC, 512], F32, tag=f"S{m}", name=f"S{m}")
                         for m in range(MT)]

                    # ---------- P2: S = B @ X^T, streaming X^T (2MB chunks) ----------
                    with (
                        tc.tile_pool(name="xts", bufs=3) as xts,
                        tc.tile_pool(name="ps2", bufs=5, space=bass.MemorySpace.PSUM) as ps2,
                        tc.tile_pool(name="psW", bufs=1, space=bass.MemorySpace.PSUM) as psW,
                    ):
                        warmt = psW.tile([128, 128], F32, tag="warm", name="warm")
                        for jc in range(JC):
                            xtc = xts.tile([128, DT, 512], F32R, tag="xtc", name="xtc")
                            nc.sync.dma_start(
                                xtc[:],
                                xt_full[:, jc * 512:(jc + 1) * 512]
                                .rearrange("(b p) j -> p b j", p=128).bitcast(F32R))
                            for m in range(MT):
                                ps = ps2.tile([128, 512], F32, tag="ps", name="ps")
                                for b in range(DT):
                                    nc.tensor.matmul(
                                        ps[:], BT[b][:, m * 128:(m + 1) * 128],
                                        xtc[:, b, :],
                                        start=(b == 0), stop=(b == DT - 1))
                                if jc == JC - 1:
                                    nc.vector.reduce_max(pmax[m][:, jc:jc + 1], ps[:],
                                                         axis=mybir.AxisListType.X)
                                    nc.vector.tensor_copy(S[m][:, jc, :], ps[:])
                                else:
                                    nc.scalar.activation(S[m][:, jc, :], ps[:], COPY)
                                    nc.vector.reduce_max(pmax[m][:, jc:jc + 1], ps[:],
                                                         axis=mybir.AxisListType.X)
                            if jc < 2:
                                for w in range(8):
                                    nc.tensor.transpose(warmt[:], ident[:], ident[:])

                    # ---------- P3: softmax + P^T + PV pass A (m=0,1) ----------
                    with (
                        tc.tile_pool(name="psW2", bufs=1, space=bass.MemorySpace.PSUM) as psW2,
                        tc.tile_pool(name="ps3", bufs=3, space=bass.MemorySpace.PSUM) as ps3,
                        tc.tile_pool(name="vtsA", bufs=3) as vtsA,
                        tc.tile_pool(name="ps4a", bufs=1, space=bass.MemorySpace.PSUM) as ps4a,
                        tc.tile_pool(name="obA", bufs=2) as obA,
                    ):
                        ppva = [[ps4a.tile([128, 512], F32, tag=f"pva{m}_{h}", name=f"pva{m}_{h}")
                                 for h in range(2)] for m in range(2)]
                        warm2 = psW2.tile([128, 128], F32, tag="warm2", name="warm2")
                        # fill the exp-wait bubble at the S->softmax seam; the pad
                        # bank must not alias the S-phase psum pools or the pads
                        # get gated on their release
                        for w in range(36):
                            nc.tensor.transpose(warm2[:], ident[:], ident[:])
                        for m in range(MT):
                            rowmax = stats.tile([128, 1], F32, tag=f"rmax{m}", name=f"rmax{m}")
                            nc.vector.reduce_max(rowmax[:], pmax[m][:], axis=mybir.AxisListType.X)
                            negb = stats.tile([128, 1], F32, tag=f"negb{m}", name=f"negb{m}")
                            nc.vector.tensor_scalar_mul(negb[:], rowmax[:], -SCALE)
                            pbig = pch.tile([128, JC * 512], F32, tag="pbig", name="pbig")
                            for hf in range(2):
                                nc.scalar.activation(
                                    pbig[:, hf * 2048:(hf + 1) * 2048],
                                    S[m][:, hf * 4:(hf + 1) * 4, :], EXP,
                                    bias=negb[:], scale=SCALE,
                                    accum_out=esum[m][:, hf:hf + 1])
                            for jc in range(JC):
                                ptps = ps3.tile([128, 4, 128], F32, tag="ptps", name="ptps")
                                for js in range(4):
                                    nc.tensor.transpose(
                                        ptps[:, js, :],
                                        pbig[:, (jc * 4 + js) * 128:(jc * 4 + js + 1) * 128],
                                        ident[:])
                                nc.vector.tensor_copy(
                                    PTm[m][:, jc * 4:(jc + 1) * 4, :], ptps[:])
                            nc.vector.reduce_sum(rsum[m][:], esum[m][:],
                                                 axis=mybir.AxisListType.X)
                            nc.vector.reciprocal(recip[:, m:m + 1], rsum[m][:])

                        # PV pass A: m=0,1 — overlaps softmax/transposes of m=2,3
                        NG = JT // 8
                        for g in range(NG):
                            vt = vtsA.tile([128, 8, D], F16, tag="vt", name="vt")
                            nc.sync.dma_start(
                                vt[:],
                                xf16[g * 1024:(g + 1) * 1024, :].rearrange("(j p) d -> p j d", p=128))
                            if g < NG - 1:
                                for j in range(8):
                                    jt = g * 8 + j
                                    for m in range(2):
                                        for h in range(2):
                                            nc.tensor.matmul(ppva[m][h][:], PTm[m][:, jt, :],
                                                             vt[:, j, h * 512:(h + 1) * 512],
                                                             start=(jt == 0), stop=False)
                            else:
                                for m in range(2):
                                    for j in range(8):
                                        jt = g * 8 + j
                                        for h in range(2):
                                            nc.tensor.matmul(ppva[m][h][:], PTm[m][:, jt, :],
                                                             vt[:, j, h * 512:(h + 1) * 512],
                                                             start=False, stop=(jt == JT - 1))
                        for m in range(2):
                            osb = obA.tile([128, D], F32, tag="osb", name="osb")
                            for h in range(2):
                                nc.vector.tensor_scalar_mul(
                                    osb[:, h * 512:(h + 1) * 512], ppva[m][h][:], recip[:, m:m + 1])
                            nc.sync.dma_start(out_l[m * 128:(m + 1) * 128, :], osb[:])

                # ---------- P4: PV pass B (m=2,3) ----------
                with (
                    tc.tile_pool(name="vtsB", bufs=3) as vtsB,
                    tc.tile_pool(name="ps4b", bufs=1, space=bass.MemorySpace.PSUM) as ps4b,
                    tc.tile_pool(name="obB", bufs=2) as obB,
                ):
                    ppvb = [[ps4b.tile([128, 512], F32, tag=f"pvb{m}_{h}", name=f"pvb{m}_{h}")
                             for h in range(2)] for m in range(2)]
                    for w in range(20):
                        nc.tensor.transpose(ppvb[0][0][:, 0:128], ident[:], ident[:])
                    NG = JT // 8
                    for g in range(NG):
                        vt = vtsB.tile([128, 8, D], F16, tag="vt", name="vt")
                        nc.sync.dma_start(
                            vt[:],
                            xf16[g * 1024:(g + 1) * 1024, :].rearrange("(j p) d -> p j d", p=128))
                        if g < NG - 1:
                            for j in range(8):
                                jt = g * 8 + j
                                for mi in range(2):
                                    for h in range(2):
                                        nc.tensor.matmul(ppvb[mi][h][:], PTm[mi + 2][:, jt, :],
                                                         vt[:, j, h * 512:(h + 1) * 512],
                                                         start=(jt == 0), stop=False)
                        else:
                            for mi in range(2):
                                for j in range(8):
                                    jt = g * 8 + j
                                    for h in range(2):
                                        nc.tensor.matmul(ppvb[mi][h][:], PTm[mi + 2][:, jt, :],
                                                         vt[:, j, h * 512:(h + 1) * 512],
                                                         start=False, stop=(jt == JT - 1))
                                m = mi + 2
                                osb = obB.tile([128, D], F32, tag="osb", name="osb")
                                for h in range(2):
                                    nc.vector.tensor_scalar_mul(
                                        osb[:, h * 512:(h + 1) * 512], ppvb[mi][h][:],
                                        recip[:, m:m + 1])
                                nc.sync.dma_start(out_l[m * 128:(m + 1) * 128, :], osb[:])

    nc.compile()
    return nc


_NC_CACHE = None


def kernel(inputs, rotation_params, entangle_params):
    global _NC_CACHE
    if _NC_CACHE is None:
        _NC_CACHE = build_nc()
    nc = _NC_CACHE
    x = np.ascontiguousarray(np.asarray(inputs, np.float32))
    wq = np.ascontiguousarray(np.asarray(rotation_params, np.float32))
    wkt = np.ascontiguousarray(np.asarray(entangle_params, np.float32).T)
    xt = np.ascontiguousarray(x.T)
    xf16_np = np.ascontiguousarray(x.astype(np.float16))
    in_maps = [
        {"xf16": xf16_np, "xt_full": xt,
         "xlt": np.ascontiguousarray(xt[:, c * NL:(c + 1) * NL]),
         "wq": wq, "wkt": wkt}
        for c in range(NC)
    ]
    r = run_bass_kernel_spmd(nc, in_maps, list(range(NC)))
    return np.concatenate([r.results[c]["out_local"] for c in range(NC)], axis=0)



# revision 7
# speedup vs baseline: 1.2250x; 1.2250x over previous
"""ClassicalSelfAttention TRN2 kernel — 8-core SPMD, sequence-parallel.

out = softmax((X Wq)(X Wk)^T / sqrt(d)) @ X,  X:[4096,1024] f32, W:[1024,1024].

Per core (rows sharded 8x512):
  host:  W = Wq @ Wk^T (weight fusion — halves projection work and traffic),
         X^T f32 rotated per core so chunk 0 is the core's own column block
         (doubles as the projection rhs — no separate xlt load), X cast to
         fp8e4 twice: x8 = e4m3(X) and dx8 = e4m3(X - x8), rolled to match
         the rotated column order.
  P1:    B^T[d,m] = sum_e W[e,d] Xl^T[e,m], 64 f32r matmuls in 2 psum waves.
  S:     S[m,j] = sum_d B^T[d,m] X^T[d,j], streaming 2MB chunks, f32r
         (1 cyc/row at 512 free). Per (m,chunk): psum->SBUF copy on Act,
         running row-max on DVE.
  P3:    per m: exp((S - max)/32) -> fp16 (Act, rowsum via accum_out),
         PE-transpose to P^T (fp16 identity: 1 cyc/row), psum->SBUF cast to
         fp8 on gpsimd.
  PV:    DoubleRow fp8 matmuls (0.5 cyc/row, 256-contraction per instr):
         out = P8 @ (x8 + dx8) accumulated across two sweeps in the same
         psum banks; double-fp8 keeps X at ~fp16 precision while the
         dominant weight (exp(0)=1.0) is exact in fp8. Scale by 1/rowsum
         (DVE) on the way out.

Schedule: T(m)=transposes, s1/s2 = x8/dx8 sweeps; order
T0 s1(0) T1 s1(1) s2(0) s2(1) T2 s1(2) T3 s1(3) s2(2) s2(3) keeps dx8's
DMA ahead of its consumers and needs only 4 psum banks + transpose scratch.
fp32r logit noise (rms ~3.4 on |S|~3e4 measured on HW) dominates rel err;
fp8 PV adds ~2e-3.
"""
import numpy as np
import ml_dtypes
import concourse.bass as bass
import concourse.bacc as bacc
import concourse.mybir as mybir
import concourse.tile as tile
from concourse import masks
from concourse.bass_utils import run_bass_kernel_spmd

F32 = mybir.dt.float32
F32R = mybir.dt.float32r
F16 = mybir.dt.float16
F8 = mybir.dt.float8e4
E4M3 = ml_dtypes.float8_e4m3
DR = mybir.MatmulPerfMode.DoubleRow
AX = mybir.AxisListType
EXP = mybir.ActivationFunctionType.Exp
COPY = mybir.ActivationFunctionType.Copy

D = 1024          # embed dim
NT = 4096         # tokens
NC = 8            # cores
NL = NT // NC     # 512 local rows
DT = D // 128     # 8 d-tiles
MT = NL // 128    # 4 m-tiles
JC = NT // 512    # 8 j-chunks
JB = NT // 256    # 16 DoubleRow j-blocks
JT = NT // 128    # 32 j-tiles
SCALE = float(1.0 / np.sqrt(np.float32(D)))


def build_nc():
    nc = bacc.Bacc("TRN2", target_bir_lowering=False, debug=False)

    w_full = nc.declare_dram_parameter("w_full", [D, D], F32, isOutput=False)
    xt_rot = nc.declare_dram_parameter("xt_rot", [D, NT], F32, isOutput=False)
    x8d = nc.declare_dram_parameter("x8d", [NT, D], F8, isOutput=False)
    dx8d = nc.declare_dram_parameter("dx8d", [NT, D], F8, isOutput=False)
    out_l = nc.declare_dram_parameter("out_local", [NL, D], F32, isOutput=True)

    with tile.TileContext(nc) as tc:
        with (
            tc.tile_pool(name="persist", bufs=1) as persist,
            tc.tile_pool(name="stats", bufs=1) as stats,
        ):
            identf = persist.tile([128, 128], F32, tag="idf", name="idf")
            masks.make_identity(nc, identf[:])
            ident16 = persist.tile([128, 128], F16, tag="id16", name="id16")
            nc.vector.tensor_copy(ident16[:], identf[:])
            BT = [persist.tile([128, NL], F32R, tag=f"BT{b}", name=f"BT{b}")
                  for b in range(DT)]
            pmax = [stats.tile([128, JC], F32, tag=f"pmax{m}", name=f"pmax{m}")
                    for m in range(MT)]
            recip = stats.tile([128, MT], F32, tag="recip", name="recip")
            x8 = persist.tile([128, JB, 2, D], F8, tag="x8", name="x8")
            S = [persist.tile([128, JC, 512], F32, tag=f"S{m}", name=f"S{m}")
                 for m in range(MT)]
            PT8 = [persist.tile([128, JT, 128], F8, tag=f"PT{m}", name=f"PT{m}")
                   for m in range(MT)]

            with tc.tile_pool(name="xts", bufs=2) as xts:
                xtc0 = xts.tile([128, DT, 512], F32R, tag="xtc", name="xtc0")
                with (
                    tc.tile_pool(name="p1", bufs=1) as p1,
                    tc.tile_pool(name="psW", bufs=1,
                                 space=bass.MemorySpace.PSUM) as psW,
                    tc.tile_pool(name="ps1", bufs=1,
                                 space=bass.MemorySpace.PSUM) as ps1,
                ):
                    # ---- P1: own chunk + W per e-tile; B^T = sum_e W Xl^T ----
                    w_t = p1.tile([128, DT, D], F32R, tag="w", name="w")
                    nc.sync.dma_start(xtc0[:, 0, :],
                                      xt_rot[0:128, 0:512].bitcast(F32R))
                    nc.sync.dma_start(w_t[:, 0, :], w_full[0:128, :].bitcast(F32R))
                    for e in range(1, DT):
                        nc.sync.dma_start(
                            xtc0[:, e, :],
                            xt_rot[e * 128:(e + 1) * 128, 0:512].bitcast(F32R))
                        nc.sync.dma_start(
                            w_t[:, e, :],
                            w_full[e * 128:(e + 1) * 128, :].bitcast(F32R))

                    warm = psW.tile([128, 128], F32, tag="warm", name="warm")
                    for _ in range(12):
                        nc.tensor.transpose(warm[:], identf[:], identf[:])

                    pb = [ps1.tile([128, NL], F32, tag=f"pb{b}", name=f"pb{b}")
                          for b in range(4)]
                    for wave in range(2):
                        for e in range(DT):
                            for bi in range(4):
                                b = wave * 4 + bi
                                nc.tensor.matmul(
                                    pb[bi][:], w_t[:, e, b * 128:(b + 1) * 128],
                                    xtc0[:, e, :], start=(e == 0),
                                    stop=(e == DT - 1))
                        for bi in range(4):
                            nc.vector.tensor_copy(BT[wave * 4 + bi][:], pb[bi][:])
                    for _ in range(10):
                        nc.tensor.transpose(warm[:], identf[:], identf[:])

                # ---- S phase ----
                with tc.tile_pool(name="ps2", bufs=5,
                                  space=bass.MemorySpace.PSUM) as ps2:
                    for jc in range(JC):
                        if jc == 0:
                            xtc = xtc0
                        else:
                            xtc = xts.tile([128, DT, 512], F32R, tag="xtc",
                                           name=f"xtc{jc}")
                            nc.sync.dma_start(
                                xtc[:],
                                xt_rot[:, jc * 512:(jc + 1) * 512]
                                .rearrange("(b p) j -> p b j", p=128).bitcast(F32R))
                        # interleave the fp8 value loads with the xt stream
                        if 1 <= jc <= 4:
                            q = jc - 1
                            nc.sync.dma_start(
                                x8[:, q * 4:(q + 1) * 4, :, :],
                                x8d[q * 1024:(q + 1) * 1024, :]
                                .rearrange("(jb two p) d -> p jb two d", p=128, two=2))
                        for m in range(MT):
                            ps = ps2.tile([128, 512], F32, tag="ps", name="ps")
                            for b in range(DT):
                                nc.tensor.matmul(
                                    ps[:], BT[b][:, m * 128:(m + 1) * 128],
                                    xtc[:, b, :],
                                    start=(b == 0), stop=(b == DT - 1))
                            nc.scalar.activation(S[m][:, jc, :], ps[:], COPY)
                            nc.vector.reduce_max(pmax[m][:, jc:jc + 1], ps[:],
                                                 axis=AX.X)

            # ---- P3 + PV ----
            with (
                tc.tile_pool(name="pch", bufs=2) as pch,
                tc.tile_pool(name="dxs", bufs=4) as dxs,
                tc.tile_pool(name="ps3", bufs=3, space=bass.MemorySpace.PSUM) as ps3,
                tc.tile_pool(name="ps4", bufs=1, space=bass.MemorySpace.PSUM) as ps4,
                tc.tile_pool(name="ob", bufs=2) as ob,
            ):
                pbig = []
                for m in range(MT):
                    rowmax = stats.tile([128, 1], F32, tag=f"rmx{m}", name=f"rmx{m}")
                    nc.vector.reduce_max(rowmax[:], pmax[m][:], axis=AX.X)
                    negb = stats.tile([128, 1], F32, tag=f"ngb{m}", name=f"ngb{m}")
                    nc.vector.tensor_scalar_mul(negb[:], rowmax[:], -SCALE)
                    pb16 = pch.tile([128, JC, 512], F16, tag="pbig", name=f"pbig{m}")
                    rsum = stats.tile([128, 1], F32, tag=f"rs{m}", name=f"rs{m}")
                    nc.scalar.activation(pb16[:], S[m][:], EXP,
                                         bias=negb[:], scale=SCALE,
                                         accum_out=rsum[:])
                    nc.vector.reciprocal(recip[:, m:m + 1], rsum[:])
                    pbig.append(pb16)

                accs = [[ps4.tile([128, 512], F32, tag=f"acc{mi}_{h}",
                                  name=f"acc{mi}_{h}")
                         for h in range(2)] for mi in range(2)]

                def transposes(m):
                    pbf = pbig[m][:].rearrange("p c f -> p (c f)")
                    for g in range(8):
                        pt = ps3.tile([128, 4, 128], F16, tag="pt", name="pt")
                        for t in range(4):
                            jt = g * 4 + t
                            nc.tensor.transpose(
                                pt[:, t, :], pbf[:, jt * 128:(jt + 1) * 128],
                                ident16[:])
                        nc.vector.tensor_copy(PT8[m][:, g * 4:(g + 1) * 4, :], pt[:])

                def sweep1(m, acc):
                    for jb in range(JB):
                        for h in range(2):
                            nc.tensor.matmul(
                                acc[h][:], PT8[m][:, 2 * jb:2 * jb + 2, :],
                                x8[:, jb, :, h * 512:(h + 1) * 512],
                                start=(jb == 0), stop=False, perf_mode=DR)

                def sweep2_pair(m0, m1, a0, a1):
                    for jb in range(JB):
                        dxt = dxs.tile([128, 2, D], F8, tag="dxt", name="dxt")
                        nc.sync.dma_start(
                            dxt[:],
                            dx8d[jb * 256:(jb + 1) * 256, :]
                            .rearrange("(two p) d -> p two d", p=128))
                        for mi, (m, acc) in enumerate(((m0, a0), (m1, a1))):
                            for h in range(2):
                                nc.tensor.matmul(
                                    acc[h][:], PT8[m][:, 2 * jb:2 * jb + 2, :],
                                    dxt[:, :, h * 512:(h + 1) * 512],
                                    start=False, stop=(jb == JB - 1), perf_mode=DR)

                def out_m(m, acc):
                    osb = ob.tile([128, D], F32, tag="osb", name=f"osb{m}")
                    for h in range(2):
                        nc.vector.tensor_scalar_mul(
                            osb[:, h * 512:(h + 1) * 512], acc[h][:],
                            recip[:, m:m + 1])
                    nc.sync.dma_start(out_l[m * 128:(m + 1) * 128, :], osb[:])

                for pair in range(2):
                    m0, m1 = 2 * pair, 2 * pair + 1
                    a0, a1 = accs[0], accs[1]
                    transposes(m0)
                    sweep1(m0, a0)
                    transposes(m1)
                    sweep1(m1, a1)
                    sweep2_pair(m0, m1, a0, a1)
                    out_m(m0, a0)
                    out_m(m1, a1)

    nc.compile()
    return nc


_NC_CACHE = None


def kernel(inputs, rotation_params, entangle_params):
    global _NC_CACHE
    if _NC_CACHE is None:
        _NC_CACHE = build_nc()
    nc = _NC_CACHE
    x = np.ascontiguousarray(np.asarray(inputs, np.float32))
    wq = np.asarray(rotation_params, np.float32)
    wk = np.asarray(entangle_params, np.float32)
    w = np.ascontiguousarray(wq @ wk.T)
    xt = np.ascontiguousarray(x.T)
    x8_np = x.astype(E4M3)
    dx8_np = (x - x8_np.astype(np.float32)).astype(E4M3)
    in_maps = []
    for c in range(NC):
        xtr = np.ascontiguousarray(np.roll(xt, -c * NL, axis=1))
        x8r = np.ascontiguousarray(np.roll(x8_np, -c * NL, axis=0))
        dx8r = np.ascontiguousarray(np.roll(dx8_np, -c * NL, axis=0))
        in_maps.append({"w_full": w, "xt_rot": xtr, "x8d": x8r, "dx8d": dx8r})
    r = run_bass_kernel_spmd(nc, in_maps, list(range(NC)))
    return np.concatenate([r.results[c]["out_local"] for c in range(NC)], axis=0)


# revision 18
# speedup vs baseline: 1.4004x; 1.1432x over previous
"""ClassicalSelfAttention TRN2 kernel — 8-core SPMD, sequence-parallel.

out = softmax((X Wq)(X Wk)^T / sqrt(d)) @ X,  X:[4096,1024] f32, W:[1024,1024].

Per core (rows sharded 8x512):
  host:  W = Wq @ Wk^T (weight fusion — halves projection work and traffic),
         X^T f32 rotated per core so chunk 0 is the core's own column block
         (doubles as the projection rhs — no separate xlt load), X cast to
         fp8e4 twice: x8 = e4m3(X) and dx8 = e4m3(X - x8), rolled to match
         the rotated column order.
  P1:    B^T[d,m] = sum_e W[e,d] Xl^T[e,m], 64 f32r matmuls, e-ordered so
         each arriving W row-block is consumed immediately.
  S:     S[m,j] = sum_d B^T[d,m] X^T[d,j], streaming 2MB chunks, f32r
         (1 cyc/row at 512 free). Per (m,chunk): psum->SBUF copy on Act,
         running row-max on DVE. x8 halves interleave with the xt stream.
         At the last chunk each m's softmax head (rowmax, exp->fp16 with
         rowsum accum) is emitted inline so exp(m0) overlaps the chunk's
         remaining matmuls and the P3 seam starts with zero PE idle.
  P3/PV: per m: PE-transpose P to P^T (fp16 identity: 1 cyc/row), psum->SBUF
         cast to fp8 alternating DVE/Act; DoubleRow fp8 matmuls (0.5 cyc/row,
         256-contraction per instr) accumulate P8 @ x8 then P8 @ dx8 (dx8
         resident by then in the space freed by the xt stream buffers);
         double-fp8 keeps X at ~fp16 precision while the dominant weight
         (exp(0)=1.0) is exact in fp8. Scale by 1/rowsum (DVE) on the way out.

Pair schedule T0 s1(0) T1 s1(1) s2(0,1) T2 s1(2) T3 s1(3) s2(2,3) needs only
4 psum accumulator banks + transpose scratch and keeps every engine fed.
fp32r logit noise (rms ~3.4 on |S|~3e4 measured on HW) dominates rel err;
fp8 PV adds ~2e-3.
"""
import numpy as np
import ml_dtypes
import concourse.bass as bass
import concourse.bacc as bacc
import concourse.mybir as mybir
import concourse.tile as tile
from concourse import masks
from concourse.bass_utils import run_bass_kernel_spmd

F32 = mybir.dt.float32
F32R = mybir.dt.float32r
F16 = mybir.dt.float16
F8 = mybir.dt.float8e4
E4M3 = ml_dtypes.float8_e4m3
DR = mybir.MatmulPerfMode.DoubleRow
AX = mybir.AxisListType
EXP = mybir.ActivationFunctionType.Exp
COPY = mybir.ActivationFunctionType.Copy

D = 1024          # embed dim
NT = 4096         # tokens
NC = 8            # cores
NL = NT // NC     # 512 local rows
DT = D // 128     # 8 d-tiles
MT = NL // 128    # 4 m-tiles
JC = NT // 512    # 8 j-chunks
JB = NT // 256    # 16 DoubleRow j-blocks
JT = NT // 128    # 32 j-tiles
SCALE = float(1.0 / np.sqrt(np.float32(D)))


def build_nc():
    nc = bacc.Bacc("TRN2", target_bir_lowering=False, debug=False)

    w_full = nc.declare_dram_parameter("w_full", [D, D], F32, isOutput=False)
    xt_rot = nc.declare_dram_parameter("xt_rot", [D, NT], F32, isOutput=False)
    x8d = nc.declare_dram_parameter("x8d", [NT, D], F8, isOutput=False)
    dx8d = nc.declare_dram_parameter("dx8d", [NT, D], F8, isOutput=False)
    out_l = nc.declare_dram_parameter("out_local", [NL, D], F32, isOutput=True)

    with tile.TileContext(nc) as tc:
        with (
            tc.tile_pool(name="persist", bufs=1) as persist,
            tc.tile_pool(name="stats", bufs=1) as stats,
            tc.tile_pool(name="pch", bufs=2) as pch,
        ):
            identf = persist.tile([128, 128], F32, tag="idf", name="idf")
            masks.make_identity(nc, identf[:])
            ident16 = persist.tile([128, 128], F16, tag="id16", name="id16")
            nc.vector.tensor_copy(ident16[:], identf[:])
            BT = [persist.tile([128, NL], F32R, tag=f"BT{b}", name=f"BT{b}")
                  for b in range(DT)]
            pmax = [stats.tile([128, JC], F32, tag=f"pmax{m}", name=f"pmax{m}")
                    for m in range(MT)]
            recip = stats.tile([128, MT], F32, tag="recip", name="recip")
            x8 = persist.tile([128, JB, 2, D], F8, tag="x8", name="x8")
            PT8 = [persist.tile([128, JT, 128], F8, tag=f"PT{m}", name=f"PT{m}")
                   for m in range(MT)]
            pbig = []

            def softmax_head(m):
                rowmax = stats.tile([128, 1], F32, tag=f"rmx{m}", name=f"rmx{m}")
                nc.vector.reduce_max(rowmax[:], pmax[m][:], axis=AX.X)
                negb = stats.tile([128, 1], F32, tag=f"ngb{m}", name=f"ngb{m}")
                nc.vector.tensor_scalar_mul(negb[:], rowmax[:], -SCALE)
                pb16 = pch.tile([128, JC, 512], F16, tag="pbig", name=f"pbig{m}")
                esum = stats.tile([128, 2], F32, tag=f"es{m}", name=f"es{m}")
                for hf in range(2):
                    nc.scalar.activation(pb16[:, hf * 4:(hf + 1) * 4, :],
                                         S[m][:, hf * 4:(hf + 1) * 4, :], EXP,
                                         bias=negb[:], scale=SCALE,
                                         accum_out=esum[:, hf:hf + 1])
                rsum = stats.tile([128, 1], F32, tag=f"rs{m}", name=f"rs{m}")
                nc.vector.reduce_sum(rsum[:], esum[:], axis=AX.X)
                nc.vector.reciprocal(recip[:, m:m + 1], rsum[:])
                pbig.append(pb16)

            with tc.tile_pool(name="spool", bufs=1) as spool:
                S = [spool.tile([128, JC, 512], F32, tag=f"S{m}", name=f"S{m}")
                     for m in range(MT)]
                with tc.tile_pool(name="xts", bufs=3) as xts:
                    xtc0 = xts.tile([128, DT, 512], F32R, tag="xtc", name="xtc0")
                    with (
                        tc.tile_pool(name="p1", bufs=3) as p1,
                        tc.tile_pool(name="ps1", bufs=1,
                                     space=bass.MemorySpace.PSUM) as ps1,
                    ):
                        # ---- P1: stream W row-blocks through 3 buffers ----
                        pb = [ps1.tile([128, NL], F32, tag=f"pb{b}", name=f"pb{b}")
                              for b in range(DT)]
                        for _ in range(14):
                            nc.tensor.transpose(pb[0][:, 0:128], identf[:],
                                                identf[:])
                        wes = []
                        for e in range(DT):
                            nc.sync.dma_start(
                                xtc0[:, e, :],
                                xt_rot[e * 128:(e + 1) * 128, 0:512].bitcast(F32R))
                            we = p1.tile([128, D], F32R, tag="w", name=f"w{e}")
                            nc.sync.dma_start(
                                we[:], w_full[e * 128:(e + 1) * 128, :]
                                .bitcast(F32R))
                            wes.append(we)
                        for e in range(DT):
                            for b in range(DT):
                                nc.tensor.matmul(
                                    pb[b][:], wes[e][:, b * 128:(b + 1) * 128],
                                    xtc0[:, e, :], start=(e == 0),
                                    stop=(e == DT - 1))
                        for b in range(DT):
                            nc.vector.tensor_copy(BT[b][:], pb[b][:])
                        for _ in range(16):
                            nc.tensor.transpose(pb[1][:, 0:128], identf[:],
                                                identf[:])

                    # ---- S phase ----
                    with tc.tile_pool(name="ps2", bufs=5,
                                      space=bass.MemorySpace.PSUM) as ps2:
                        for jc in range(JC):
                            if jc == 0:
                                xtc = xtc0
                            else:
                                xtc = xts.tile([128, DT, 512], F32R, tag="xtc",
                                               name=f"xtc{jc}")
                                nc.sync.dma_start(
                                    xtc[:],
                                    xt_rot[:, jc * 512:(jc + 1) * 512]
                                    .rearrange("(b p) j -> p b j", p=128)
                                    .bitcast(F32R))
                            if 1 <= jc <= 7:
                                q = jc - 1
                                nc.sync.dma_start(
                                    x8[:, q * 2:(q + 1) * 2, :, :],
                                    x8d[q * 512:(q + 1) * 512, :]
                                    .rearrange("(jb two p) d -> p jb two d",
                                               p=128, two=2))
                            for m in range(MT):
                                ps = ps2.tile([128, 512], F32, tag="ps", name="ps")
                                for b in range(DT):
                                    nc.tensor.matmul(
                                        ps[:], BT[b][:, m * 128:(m + 1) * 128],
                                        xtc[:, b, :],
                                        start=(b == 0), stop=(b == DT - 1))
                                if jc == JC - 1:
                                    nc.vector.tensor_copy(S[m][:, jc, :], ps[:])
                                else:
                                    nc.scalar.activation(S[m][:, jc, :], ps[:],
                                                         COPY)
                                nc.vector.reduce_max(pmax[m][:, jc:jc + 1], ps[:],
                                                     axis=AX.X)
                                if jc == JC - 1:
                                    softmax_head(m)

                # xts freed: last x8 quarter + resident dx8 go into its space
                with (
                    tc.tile_pool(name="dxr", bufs=1) as dxr,
                    tc.tile_pool(name="ps3", bufs=4,
                                 space=bass.MemorySpace.PSUM) as ps3,
                    tc.tile_pool(name="ps4", bufs=1,
                                 space=bass.MemorySpace.PSUM) as ps4,
                    tc.tile_pool(name="ob", bufs=2) as ob,
                ):
                    nc.sync.dma_start(
                        x8[:, 14:16, :, :],
                        x8d[7 * 512:8 * 512, :]
                        .rearrange("(jb two p) d -> p jb two d", p=128, two=2))
                    dx8 = dxr.tile([128, JB, 2, D], F8, tag="dx8", name="dx8")
                    for q in range(4):
                        nc.sync.dma_start(
                            dx8[:, q * 4:(q + 1) * 4, :, :],
                            dx8d[q * 1024:(q + 1) * 1024, :]
                            .rearrange("(jb two p) d -> p jb two d", p=128, two=2))

                    accs = [[ps4.tile([128, 512], F32, tag=f"acc{mi}_{h}",
                                      name=f"acc{mi}_{h}")
                             for h in range(2)] for mi in range(2)]

                    def t_sweep1(m, acc):
                        pbf = pbig[m][:].rearrange("p c f -> p (c f)")

                        def mm_group(g):
                            for jb in (2 * g, 2 * g + 1):
                                for h in range(2):
                                    nc.tensor.matmul(
                                        acc[h][:], PT8[m][:, 2 * jb:2 * jb + 2, :],
                                        x8[:, jb, :, h * 512:(h + 1) * 512],
                                        start=(jb == 0), stop=False, perf_mode=DR)

                        for g in range(8):
                            pt = ps3.tile([128, 4, 128], F16, tag="pt", name="pt")
                            for t in range(4):
                                jt = g * 4 + t
                                nc.tensor.transpose(
                                    pt[:, t, :], pbf[:, jt * 128:(jt + 1) * 128],
                                    ident16[:])
                            dst = PT8[m][:, g * 4:(g + 1) * 4, :]
                            if g % 2 == 0:
                                nc.vector.tensor_copy(dst, pt[:])
                            else:
                                nc.scalar.activation(dst, pt[:], COPY)
                            if g > 0:
                                mm_group(g - 1)
                        mm_group(7)

                    def out_m(m, acc):
                        osb = ob.tile([128, D], F32, tag="osb", name=f"osb{m}")
                        nc.vector.tensor_scalar_mul(
                            osb[:, 0:512], acc[0][:], recip[:, m:m + 1])
                        nc.scalar.activation(osb[:, 512:1024], acc[1][:], COPY,
                                             scale=recip[:, m:m + 1])
                        nc.sync.dma_start(out_l[m * 128:(m + 1) * 128, :], osb[:])

                    def sweep2_pair(m0, m1, a0, a1, seq):
                        if seq:
                            for m, acc in ((m0, a0), (m1, a1)):
                                for jb in range(JB):
                                    for h in range(2):
                                        nc.tensor.matmul(
                                            acc[h][:],
                                            PT8[m][:, 2 * jb:2 * jb + 2, :],
                                            dx8[:, jb, :, h * 512:(h + 1) * 512],
                                            start=False, stop=(jb == JB - 1),
                                            perf_mode=DR)
                                out_m(m, acc)
                        else:
                            for jb in range(JB):
                                last = jb == JB - 1
                                for m, acc in ((m0, a0), (m1, a1)):
                                    for h in range(2):
                                        nc.tensor.matmul(
                                            acc[h][:],
                                            PT8[m][:, 2 * jb:2 * jb + 2, :],
                                            dx8[:, jb, :, h * 512:(h + 1) * 512],
                                            start=False, stop=last, perf_mode=DR)
                                    if last:
                                        out_m(m, acc)

                    for pair in range(2):
                        m0, m1 = 2 * pair, 2 * pair + 1
                        a0, a1 = accs[0], accs[1]
                        t_sweep1(m0, a0)
                        t_sweep1(m1, a1)
                        sweep2_pair(m0, m1, a0, a1, seq=(pair == 1))

    nc.compile()
    return nc


_NC_CACHE = None


def kernel(inputs, rotation_params, entangle_params):
    global _NC_CACHE
    if _NC_CACHE is None:
        _NC_CACHE = build_nc()
    nc = _NC_CACHE
    x = np.ascontiguousarray(np.asarray(inputs, np.float32))
    wq = np.asarray(rotation_params, np.float32)
    wk = np.asarray(entangle_params, np.float32)
    w = np.ascontiguousarray(wq @ wk.T)
    xt = np.ascontiguousarray(x.T)
    x8_np = x.astype(E4M3)
    dx8_np = (x - x8_np.astype(np.float32)).astype(E4M3)
    in_maps = []
    for c in range(NC):
        xtr = np.ascontiguousarray(np.roll(xt, -c * NL, axis=1))
        x8r = np.ascontiguousarray(np.roll(x8_np, -c * NL, axis=0))
        dx8r = np.ascontiguousarray(np.roll(dx8_np, -c * NL, axis=0))
        in_maps.append({"w_full": w, "xt_rot": xtr, "x8d": x8r, "dx8d": dx8r})
    r = run_bass_kernel_spmd(nc, in_maps, list(range(NC)))
    return np.concatenate([r.results[c]["out_local"] for c in range(NC)], axis=0)


# revision 20
# speedup vs baseline: 1.4227x; 1.0159x over previous
"""ClassicalSelfAttention TRN2 kernel — 8-core SPMD, sequence-parallel.

out = softmax((X Wq)(X Wk)^T / sqrt(d)) @ X,  X:[4096,1024] f32, W:[1024,1024].

Per core (rows sharded 8x512):
  host:  W = Wq @ Wk^T (weight fusion — halves projection work and traffic),
         X^T f32 rotated per core so chunk 0 is the core's own column block
         (doubles as the projection rhs — no separate xlt load), X cast to
         fp8e4 twice: x8 = e4m3(X) and dx8 = e4m3(X - x8), rolled to match
         the rotated column order.
  P1:    B^T[d,m] = sum_e W[e,d] Xl^T[e,m], 64 f32r matmuls, e-ordered so
         each arriving W row-block is consumed immediately.
  S:     S[m,j] = sum_d B^T[d,m] X^T[d,j], streaming 2MB chunks, f32r
         (1 cyc/row at 512 free). Per (m,chunk): psum->SBUF copy on Act,
         running row-max on DVE. x8 halves interleave with the xt stream.
         At the last chunk each m's softmax head (rowmax, exp->fp16 with
         rowsum accum) is emitted inline so exp(m0) overlaps the chunk's
         remaining matmuls and the P3 seam starts with zero PE idle.
  P3/PV: per m: PE-transpose P to P^T (fp16 identity: 1 cyc/row), psum->SBUF
         cast to fp8 alternating DVE/Act; DoubleRow fp8 matmuls (0.5 cyc/row,
         256-contraction per instr) accumulate P8 @ x8 then P8 @ dx8 (dx8
         resident by then in the space freed by the xt stream buffers);
         double-fp8 keeps X at ~fp16 precision while the dominant weight
         (exp(0)=1.0) is exact in fp8. Scale by 1/rowsum (DVE) on the way out.

Pair schedule T0 s1(0) T1 s1(1) s2(0,1) T2 s1(2) T3 s1(3) s2(2,3) needs only
4 psum accumulator banks + transpose scratch and keeps every engine fed.
fp32r logit noise (rms ~3.4 on |S|~3e4 measured on HW) dominates rel err;
fp8 PV adds ~2e-3.
"""
import numpy as np
import ml_dtypes
import concourse.bass as bass
import concourse.bacc as bacc
import concourse.mybir as mybir
import concourse.tile as tile
from concourse import masks
from concourse.bass_utils import run_bass_kernel_spmd

F32 = mybir.dt.float32
F32R = mybir.dt.float32r
F16 = mybir.dt.float16
F8 = mybir.dt.float8e4
E4M3 = ml_dtypes.float8_e4m3
DR = mybir.MatmulPerfMode.DoubleRow
AX = mybir.AxisListType
EXP = mybir.ActivationFunctionType.Exp
COPY = mybir.ActivationFunctionType.Copy

D = 1024          # embed dim
NT = 4096         # tokens
NC = 8            # cores
NL = NT // NC     # 512 local rows
DT = D // 128     # 8 d-tiles
MT = NL // 128    # 4 m-tiles
JC = NT // 512    # 8 j-chunks
JB = NT // 256    # 16 DoubleRow j-blocks
JT = NT // 128    # 32 j-tiles
SCALE = float(1.0 / np.sqrt(np.float32(D)))


def build_nc():
    nc = bacc.Bacc("TRN2", target_bir_lowering=False, debug=False)

    w_full = nc.declare_dram_parameter("w_full", [D, D], F32, isOutput=False)
    xt_rot = nc.declare_dram_parameter("xt_rot", [D, NT], F32, isOutput=False)
    x8d = nc.declare_dram_parameter("x8d", [NT, D], F8, isOutput=False)
    dx8d = nc.declare_dram_parameter("dx8d", [NT, D], F8, isOutput=False)
    out_l = nc.declare_dram_parameter("out_local", [NL, D], F32, isOutput=True)

    with tile.TileContext(nc) as tc:
        with (
            tc.tile_pool(name="persist", bufs=1) as persist,
            tc.tile_pool(name="stats", bufs=1) as stats,
            tc.tile_pool(name="pch", bufs=2) as pch,
        ):
            identf = persist.tile([128, 128], F32, tag="idf", name="idf")
            masks.make_identity(nc, identf[:])
            ident16 = persist.tile([128, 128], F16, tag="id16", name="id16")
            nc.vector.tensor_copy(ident16[:], identf[:])
            BT = [persist.tile([128, NL], F32R, tag=f"BT{b}", name=f"BT{b}")
                  for b in range(DT)]
            pmax = [stats.tile([128, JC], F32, tag=f"pmax{m}", name=f"pmax{m}")
                    for m in range(MT)]
            recip = stats.tile([128, MT], F32, tag="recip", name="recip")
            x8 = persist.tile([128, JB, 2, D], F8, tag="x8", name="x8")
            PT8 = [persist.tile([128, JT, 128], F8, tag=f"PT{m}", name=f"PT{m}")
                   for m in range(MT)]
            pbig = []

            def softmax_head(m):
                rowmax = stats.tile([128, 1], F32, tag=f"rmx{m}", name=f"rmx{m}")
                nc.vector.reduce_max(rowmax[:], pmax[m][:], axis=AX.X)
                negb = stats.tile([128, 1], F32, tag=f"ngb{m}", name=f"ngb{m}")
                nc.vector.tensor_scalar_mul(negb[:], rowmax[:], -SCALE)
                pb16 = pch.tile([128, JC, 512], F16, tag="pbig", name=f"pbig{m}")
                esum = stats.tile([128, 2], F32, tag=f"es{m}", name=f"es{m}")
                for hf in range(2):
                    nc.scalar.activation(pb16[:, hf * 4:(hf + 1) * 4, :],
                                         S[m][:, hf * 4:(hf + 1) * 4, :], EXP,
                                         bias=negb[:], scale=SCALE,
                                         accum_out=esum[:, hf:hf + 1])
                rsum = stats.tile([128, 1], F32, tag=f"rs{m}", name=f"rs{m}")
                nc.vector.reduce_sum(rsum[:], esum[:], axis=AX.X)
                nc.vector.reciprocal(recip[:, m:m + 1], rsum[:])
                pbig.append(pb16)

            with tc.tile_pool(name="spool", bufs=1) as spool:
                S = [spool.tile([128, JC, 512], F32, tag=f"S{m}", name=f"S{m}")
                     for m in range(MT)]
                with tc.tile_pool(name="xts", bufs=3) as xts:
                    xtc0 = xts.tile([128, DT, 512], F32R, tag="xtc", name="xtc0")
                    with (
                        tc.tile_pool(name="p1", bufs=3) as p1,
                        tc.tile_pool(name="ps1", bufs=1,
                                     space=bass.MemorySpace.PSUM) as ps1,
                    ):
                        # ---- P1: stream W row-blocks through 3 buffers ----
                        pb = [ps1.tile([128, NL], F32, tag=f"pb{b}", name=f"pb{b}")
                              for b in range(DT)]
                        for _ in range(14):
                            nc.tensor.transpose(pb[0][:, 0:128], identf[:],
                                                identf[:])
                        wes = []
                        for e in range(DT):
                            nc.sync.dma_start(
                                xtc0[:, e, :],
                                xt_rot[e * 128:(e + 1) * 128, 0:512].bitcast(F32R))
                            we = p1.tile([128, D], F32R, tag="w", name=f"w{e}")
                            nc.sync.dma_start(
                                we[:], w_full[e * 128:(e + 1) * 128, :]
                                .bitcast(F32R))
                            wes.append(we)
                        for e in range(DT):
                            for b in range(DT):
                                nc.tensor.matmul(
                                    pb[b][:], wes[e][:, b * 128:(b + 1) * 128],
                                    xtc0[:, e, :], start=(e == 0),
                                    stop=(e == DT - 1))
                        for b in range(DT):
                            if b % 2 == 0:
                                nc.vector.tensor_copy(BT[b][:], pb[b][:])
                            else:
                                nc.scalar.activation(BT[b][:], pb[b][:], COPY)
                        for _ in range(16):
                            nc.tensor.transpose(pb[1][:, 0:128], identf[:],
                                                identf[:])

                    # ---- S phase ----
                    with tc.tile_pool(name="ps2", bufs=5,
                                      space=bass.MemorySpace.PSUM) as ps2:
                        for jc in range(JC):
                            if jc == 0:
                                xtc = xtc0
                            else:
                                xtc = xts.tile([128, DT, 512], F32R, tag="xtc",
                                               name=f"xtc{jc}")
                                nc.sync.dma_start(
                                    xtc[:],
                                    xt_rot[:, jc * 512:(jc + 1) * 512]
                                    .rearrange("(b p) j -> p b j", p=128)
                                    .bitcast(F32R))
                            if 1 <= jc <= 7:
                                q = jc - 1
                                nc.sync.dma_start(
                                    x8[:, q * 2:(q + 1) * 2, :, :],
                                    x8d[q * 512:(q + 1) * 512, :]
                                    .rearrange("(jb two p) d -> p jb two d",
                                               p=128, two=2))
                            for m in range(MT):
                                ps = ps2.tile([128, 512], F32, tag="ps", name="ps")
                                for b in range(DT):
                                    nc.tensor.matmul(
                                        ps[:], BT[b][:, m * 128:(m + 1) * 128],
                                        xtc[:, b, :],
                                        start=(b == 0), stop=(b == DT - 1))
                                if jc == JC - 1:
                                    nc.vector.tensor_copy(S[m][:, jc, :], ps[:])
                                else:
                                    nc.scalar.activation(S[m][:, jc, :], ps[:],
                                                         COPY)
                                nc.vector.reduce_max(pmax[m][:, jc:jc + 1], ps[:],
                                                     axis=AX.X)
                                if jc == JC - 1:
                                    softmax_head(m)

                # xts freed: last x8 quarter + resident dx8 go into its space
                with (
                    tc.tile_pool(name="dxr", bufs=1) as dxr,
                    tc.tile_pool(name="psw2", bufs=1,
                                 space=bass.MemorySpace.PSUM) as psw2,
                    tc.tile_pool(name="ps3", bufs=3,
                                 space=bass.MemorySpace.PSUM) as ps3,
                    tc.tile_pool(name="ps4", bufs=1,
                                 space=bass.MemorySpace.PSUM) as ps4,
                    tc.tile_pool(name="ob", bufs=2) as ob,
                ):
                    nc.sync.dma_start(
                        x8[:, 14:16, :, :],
                        x8d[7 * 512:8 * 512, :]
                        .rearrange("(jb two p) d -> p jb two d", p=128, two=2))
                    dx8 = dxr.tile([128, JB, 2, D], F8, tag="dx8", name="dx8")
                    for q in range(8):
                        nc.sync.dma_start(
                            dx8[:, q * 2:(q + 1) * 2, :, :],
                            dx8d[q * 512:(q + 1) * 512, :]
                            .rearrange("(jb two p) d -> p jb two d", p=128, two=2))

                    accs = [[ps4.tile([128, 512], F32, tag=f"acc{mi}_{h}",
                                      name=f"acc{mi}_{h}")
                             for h in range(2)] for mi in range(2)]

                    def t_sweep1(m, acc):
                        pbf = pbig[m][:].rearrange("p c f -> p (c f)")

                        def mm_group(g):
                            for jb in (2 * g, 2 * g + 1):
                                for h in range(2):
                                    nc.tensor.matmul(
                                        acc[h][:], PT8[m][:, 2 * jb:2 * jb + 2, :],
                                        x8[:, jb, :, h * 512:(h + 1) * 512],
                                        start=(jb == 0), stop=False, perf_mode=DR)

                        for g in range(8):
                            pt = ps3.tile([128, 4, 128], F16, tag="pt", name="pt")
                            for t in range(4):
                                jt = g * 4 + t
                                nc.tensor.transpose(
                                    pt[:, t, :], pbf[:, jt * 128:(jt + 1) * 128],
                                    ident16[:])
                            dst = PT8[m][:, g * 4:(g + 1) * 4, :]
                            if g % 2 == 0:
                                nc.vector.tensor_copy(dst, pt[:])
                            else:
                                nc.scalar.activation(dst, pt[:], COPY)
                            if g > 0:
                                mm_group(g - 1)
                        mm_group(7)

                    def out_m(m, acc):
                        osb = ob.tile([128, D], F32, tag="osb", name=f"osb{m}")
                        nc.vector.tensor_scalar_mul(
                            osb[:, 0:512], acc[0][:], recip[:, m:m + 1])
                        nc.scalar.activation(osb[:, 512:1024], acc[1][:], COPY,
                                             scale=recip[:, m:m + 1])
                        nc.sync.dma_start(out_l[m * 128:(m + 1) * 128, :], osb[:])

                    def sweep2_pair(m0, m1, a0, a1, seq):
                        if seq:
                            for m, acc in ((m0, a0), (m1, a1)):
                                for jb in range(JB):
                                    for h in range(2):
                                        nc.tensor.matmul(
                                            acc[h][:],
                                            PT8[m][:, 2 * jb:2 * jb + 2, :],
                                            dx8[:, jb, :, h * 512:(h + 1) * 512],
                                            start=False, stop=(jb == JB - 1),
                                            perf_mode=DR)
                                out_m(m, acc)
                        else:
                            for jb in range(JB):
                                last = jb == JB - 1
                                for m, acc in ((m0, a0), (m1, a1)):
                                    for h in range(2):
                                        nc.tensor.matmul(
                                            acc[h][:],
                                            PT8[m][:, 2 * jb:2 * jb + 2, :],
                                            dx8[:, jb, :, h * 512:(h + 1) * 512],
                                            start=False, stop=last, perf_mode=DR)
                                    if last:
                                        out_m(m, acc)

                    warmt = psw2.tile([128, 128], F16, tag="wrm", name="wrm")
                    for pair in range(2):
                        m0, m1 = 2 * pair, 2 * pair + 1
                        a0, a1 = accs[0], accs[1]
                        if pair == 0:
                            for _ in range(24):
                                nc.tensor.transpose(warmt[:], ident16[:],
                                                    ident16[:])
                        t_sweep1(m0, a0)
                        t_sweep1(m1, a1)
                        if pair == 0:
                            for _ in range(10):
                                nc.tensor.transpose(warmt[:], ident16[:],
                                                    ident16[:])
                        sweep2_pair(m0, m1, a0, a1, seq=(pair == 1))

    nc.compile()
    return nc


_NC_CACHE = None


def kernel(inputs, rotation_params, entangle_params):
    global _NC_CACHE
    if _NC_CACHE is None:
        _NC_CACHE = build_nc()
    nc = _NC_CACHE
    x = np.ascontiguousarray(np.asarray(inputs, np.float32))
    wq = np.asarray(rotation_params, np.float32)
    wk = np.asarray(entangle_params, np.float32)
    w = np.ascontiguousarray(wq @ wk.T)
    xt = np.ascontiguousarray(x.T)
    x8_np = x.astype(E4M3)
    dx8_np = (x - x8_np.astype(np.float32)).astype(E4M3)
    in_maps = []
    for c in range(NC):
        xtr = np.ascontiguousarray(np.roll(xt, -c * NL, axis=1))
        x8r = np.ascontiguousarray(np.roll(x8_np, -c * NL, axis=0))
        dx8r = np.ascontiguousarray(np.roll(dx8_np, -c * NL, axis=0))
        in_maps.append({"w_full": w, "xt_rot": xtr, "x8d": x8r, "dx8d": dx8r})
    r = run_bass_kernel_spmd(nc, in_maps, list(range(NC)))
    return np.concatenate([r.results[c]["out_local"] for c in range(NC)], axis=0)


# revision 26
# speedup vs baseline: 1.4259x; 1.0022x over previous
"""ClassicalSelfAttention TRN2 kernel — 8-core SPMD, sequence-parallel.

out = softmax((X Wq)(X Wk)^T / sqrt(d)) @ X,  X:[4096,1024] f32, W:[1024,1024].

Per core (rows sharded 8x512):
  host:  W = Wq @ Wk^T (weight fusion — halves projection work and traffic),
         X^T f32 rotated per core so chunk 0 is the core's own column block
         (doubles as the projection rhs — no separate xlt load), X cast to
         fp8e4 twice: x8 = e4m3(X) and dx8 = e4m3(X - x8), rolled to match
         the rotated column order.
  P1:    B^T[d,m] = sum_e W[e,d] Xl^T[e,m], 64 f32r matmuls, e-ordered so
         each arriving W row-block is consumed immediately.
  S:     S[m,j] = sum_d B^T[d,m] X^T[d,j], streaming 2MB chunks, f32r
         (1 cyc/row at 512 free). Per (m,chunk): psum->SBUF copy on Act,
         running row-max on DVE. x8 halves interleave with the xt stream.
         At the last chunk each m's softmax head (rowmax, exp->fp16 with
         rowsum accum) is emitted inline so exp(m0) overlaps the chunk's
         remaining matmuls and the P3 seam starts with zero PE idle.
  P3/PV: per m: PE-transpose P to P^T (fp16 identity: 1 cyc/row), psum->SBUF
         cast to fp8 alternating DVE/Act; DoubleRow fp8 matmuls (0.5 cyc/row,
         256-contraction per instr) accumulate P8 @ x8 then P8 @ dx8 (dx8
         resident by then in the space freed by the xt stream buffers);
         double-fp8 keeps X at ~fp16 precision while the dominant weight
         (exp(0)=1.0) is exact in fp8. Scale by 1/rowsum (DVE) on the way out.

Pair schedule T0 s1(0) T1 s1(1) s2(0,1) T2 s1(2) T3 s1(3) s2(2,3) needs only
4 psum accumulator banks + transpose scratch and keeps every engine fed.
fp32r logit noise (rms ~3.4 on |S|~3e4 measured on HW) dominates rel err;
fp8 PV adds ~2e-3.
"""
import numpy as np
import ml_dtypes
import concourse.bass as bass
import concourse.bacc as bacc
import concourse.mybir as mybir
import concourse.tile as tile
from concourse import masks
from concourse.bass_utils import run_bass_kernel_spmd

F32 = mybir.dt.float32
F32R = mybir.dt.float32r
F16 = mybir.dt.float16
F8 = mybir.dt.float8e4
E4M3 = ml_dtypes.float8_e4m3
DR = mybir.MatmulPerfMode.DoubleRow
AX = mybir.AxisListType
EXP = mybir.ActivationFunctionType.Exp
COPY = mybir.ActivationFunctionType.Copy

D = 1024          # embed dim
NT = 4096         # tokens
NC = 8            # cores
NL = NT // NC     # 512 local rows
DT = D // 128     # 8 d-tiles
MT = NL // 128    # 4 m-tiles
JC = NT // 512    # 8 j-chunks
JB = NT // 256    # 16 DoubleRow j-blocks
JT = NT // 128    # 32 j-tiles
SCALE = float(1.0 / np.sqrt(np.float32(D)))


def build_nc():
    nc = bacc.Bacc("TRN2", target_bir_lowering=False, debug=False)

    w_full = nc.declare_dram_parameter("w_full", [D, D], F32, isOutput=False)
    xt_rot = nc.declare_dram_parameter("xt_rot", [D, NT], F32, isOutput=False)
    x8d = nc.declare_dram_parameter("x8d", [NT, D], F8, isOutput=False)
    dx8d = nc.declare_dram_parameter("dx8d", [NT, D], F8, isOutput=False)
    out_l = nc.declare_dram_parameter("out_local", [NL, D], F32, isOutput=True)

    with tile.TileContext(nc) as tc:
        with (
            tc.tile_pool(name="persist", bufs=1) as persist,
            tc.tile_pool(name="stats", bufs=1) as stats,
            tc.tile_pool(name="pch", bufs=2) as pch,
        ):
            identf = persist.tile([128, 128], F32, tag="idf", name="idf")
            masks.make_identity(nc, identf[:])
            ident16 = persist.tile([128, 128], F16, tag="id16", name="id16")
            nc.vector.tensor_copy(ident16[:], identf[:])
            BT = [persist.tile([128, NL], F32R, tag=f"BT{b}", name=f"BT{b}")
                  for b in range(DT)]
            pmax = [stats.tile([128, JC], F32, tag=f"pmax{m}", name=f"pmax{m}")
                    for m in range(MT)]
            recip = stats.tile([128, MT], F32, tag="recip", name="recip")
            x8 = persist.tile([128, JB, 2, D], F8, tag="x8", name="x8")
            PT8 = [persist.tile([128, JT, 128], F8, tag=f"PT{m}", name=f"PT{m}")
                   for m in range(MT)]
            pbig = []

            def softmax_head(m):
                rowmax = stats.tile([128, 1], F32, tag=f"rmx{m}", name=f"rmx{m}")
                nc.vector.reduce_max(rowmax[:], pmax[m][:], axis=AX.X)
                negb = stats.tile([128, 1], F32, tag=f"ngb{m}", name=f"ngb{m}")
                nc.vector.tensor_scalar_mul(negb[:], rowmax[:], -SCALE)
                pb16 = pch.tile([128, JC, 512], F16, tag="pbig", name=f"pbig{m}")
                esum = stats.tile([128, 2], F32, tag=f"es{m}", name=f"es{m}")
                for hf in range(2):
                    nc.scalar.activation(pb16[:, hf * 4:(hf + 1) * 4, :],
                                         S[m][:, hf * 4:(hf + 1) * 4, :], EXP,
                                         bias=negb[:], scale=SCALE,
                                         accum_out=esum[:, hf:hf + 1])
                rsum = stats.tile([128, 1], F32, tag=f"rs{m}", name=f"rs{m}")
                nc.vector.reduce_sum(rsum[:], esum[:], axis=AX.X)
                nc.vector.reciprocal(recip[:, m:m + 1], rsum[:])
                pbig.append(pb16)

            with tc.tile_pool(name="spool", bufs=1) as spool:
                S = [spool.tile([128, JC, 512], F32, tag=f"S{m}", name=f"S{m}")
                     for m in range(MT)]
                with tc.tile_pool(name="xts", bufs=3) as xts:
                    xtc0 = xts.tile([128, DT, 512], F32R, tag="xtc", name="xtc0")
                    with (
                        tc.tile_pool(name="p1", bufs=3) as p1,
                        tc.tile_pool(name="ps1", bufs=1,
                                     space=bass.MemorySpace.PSUM) as ps1,
                    ):
                        # ---- P1: stream W row-blocks through 3 buffers ----
                        pb = [ps1.tile([128, NL], F32, tag=f"pb{b}", name=f"pb{b}")
                              for b in range(DT)]
                        for _ in range(14):
                            nc.tensor.transpose(pb[0][:, 0:128], identf[:],
                                                identf[:])
                        wes = []
                        for e in range(DT):
                            nc.sync.dma_start(
                                xtc0[:, e, :],
                                xt_rot[e * 128:(e + 1) * 128, 0:512].bitcast(F32R))
                            we = p1.tile([128, D], F32R, tag="w", name=f"w{e}")
                            nc.sync.dma_start(
                                we[:], w_full[e * 128:(e + 1) * 128, :]
                                .bitcast(F32R))
                            wes.append(we)
                        for e in range(DT):
                            for b in range(DT):
                                nc.tensor.matmul(
                                    pb[b][:], wes[e][:, b * 128:(b + 1) * 128],
                                    xtc0[:, e, :], start=(e == 0),
                                    stop=(e == DT - 1))
                        for b in range(DT):
                            if b % 2 == 0:
                                nc.vector.tensor_copy(BT[b][:], pb[b][:])
                            else:
                                nc.scalar.activation(BT[b][:], pb[b][:], COPY)
                        for _ in range(16):
                            nc.tensor.transpose(pb[1][:, 0:128], identf[:],
                                                identf[:])

                    # ---- S phase ----
                    with tc.tile_pool(name="ps2", bufs=5,
                                      space=bass.MemorySpace.PSUM) as ps2:
                        for jc in range(JC):
                            if jc == 0:
                                xtc = xtc0
                            else:
                                xtc = xts.tile([128, DT, 512], F32R, tag="xtc",
                                               name=f"xtc{jc}")
                                nc.sync.dma_start(
                                    xtc[:],
                                    xt_rot[:, jc * 512:(jc + 1) * 512]
                                    .rearrange("(b p) j -> p b j", p=128)
                                    .bitcast(F32R))
                            if 1 <= jc <= 7:
                                q = jc - 1
                                nc.sync.dma_start(
                                    x8[:, q * 2:(q + 1) * 2, :, :],
                                    x8d[q * 512:(q + 1) * 512, :]
                                    .rearrange("(jb two p) d -> p jb two d",
                                               p=128, two=2))
                            for m in range(MT):
                                ps = ps2.tile([128, 512], F32, tag="ps", name="ps")
                                for b in range(DT):
                                    nc.tensor.matmul(
                                        ps[:], BT[b][:, m * 128:(m + 1) * 128],
                                        xtc[:, b, :],
                                        start=(b == 0), stop=(b == DT - 1))
                                if jc == JC - 1:
                                    nc.vector.tensor_copy(S[m][:, jc, :], ps[:])
                                else:
                                    nc.scalar.activation(S[m][:, jc, :], ps[:],
                                                         COPY)
                                nc.vector.reduce_max(pmax[m][:, jc:jc + 1], ps[:],
                                                     axis=AX.X)
                                if jc == JC - 1:
                                    softmax_head(m)

                # xts freed: last x8 quarter + resident dx8 go into its space
                with (
                    tc.tile_pool(name="dxr", bufs=1) as dxr,
                    tc.tile_pool(name="psw2", bufs=1,
                                 space=bass.MemorySpace.PSUM) as psw2,
                    tc.tile_pool(name="ps3", bufs=3,
                                 space=bass.MemorySpace.PSUM) as ps3,
                    tc.tile_pool(name="ps4", bufs=1,
                                 space=bass.MemorySpace.PSUM) as ps4,
                    tc.tile_pool(name="ob", bufs=2) as ob,
                ):
                    dx8 = dxr.tile([128, JB, 2, D], F8, tag="dx8", name="dx8")
                    nc.sync.dma_start(
                        dx8[:, 0:2, :, :],
                        dx8d[0:512, :]
                        .rearrange("(jb two p) d -> p jb two d", p=128, two=2))
                    nc.sync.dma_start(
                        x8[:, 14:16, :, :],
                        x8d[7 * 512:8 * 512, :]
                        .rearrange("(jb two p) d -> p jb two d", p=128, two=2))
                    for q in range(1, 8):
                        nc.sync.dma_start(
                            dx8[:, q * 2:(q + 1) * 2, :, :],
                            dx8d[q * 512:(q + 1) * 512, :]
                            .rearrange("(jb two p) d -> p jb two d", p=128, two=2))

                    accs = [[ps4.tile([128, 512], F32, tag=f"acc{mi}_{h}",
                                      name=f"acc{mi}_{h}")
                             for h in range(2)] for mi in range(2)]

                    def t_sweep1(m, acc):
                        pbf = pbig[m][:].rearrange("p c f -> p (c f)")

                        def mm_group(g):
                            for jb in (2 * g, 2 * g + 1):
                                for h in range(2):
                                    nc.tensor.matmul(
                                        acc[h][:], PT8[m][:, 2 * jb:2 * jb + 2, :],
                                        x8[:, jb, :, h * 512:(h + 1) * 512],
                                        start=(jb == 0), stop=False, perf_mode=DR)

                        for g in range(8):
                            pt = ps3.tile([128, 4, 128], F16, tag="pt", name="pt")
                            for t in range(4):
                                jt = g * 4 + t
                                nc.tensor.transpose(
                                    pt[:, t, :], pbf[:, jt * 128:(jt + 1) * 128],
                                    ident16[:])
                            dst = PT8[m][:, g * 4:(g + 1) * 4, :]
                            if g % 2 == 0:
                                nc.vector.tensor_copy(dst, pt[:])
                            else:
                                nc.scalar.activation(dst, pt[:], COPY)
                            if g > 1:
                                mm_group(g - 2)
                        mm_group(6)
                        mm_group(7)

                    def out_m(m, acc):
                        osb = ob.tile([128, D], F32, tag="osb", name=f"osb{m}")
                        nc.vector.tensor_scalar_mul(
                            osb[:, 0:512], acc[0][:], recip[:, m:m + 1])
                        nc.scalar.activation(osb[:, 512:1024], acc[1][:], COPY,
                                             scale=recip[:, m:m + 1])
                        nc.sync.dma_start(out_l[m * 128:(m + 1) * 128, :], osb[:])

                    def sweep2_pair(m0, m1, a0, a1, seq):
                        if seq:
                            for m, acc in ((m0, a0), (m1, a1)):
                                for jb in range(JB):
                                    for h in range(2):
                                        nc.tensor.matmul(
                                            acc[h][:],
                                            PT8[m][:, 2 * jb:2 * jb + 2, :],
                                            dx8[:, jb, :, h * 512:(h + 1) * 512],
                                            start=False, stop=(jb == JB - 1),
                                            perf_mode=DR)
                                out_m(m, acc)
                        else:
                            for jb in range(JB):
                                last = jb == JB - 1
                                for m, acc in ((m0, a0), (m1, a1)):
                                    for h in range(2):
                                        nc.tensor.matmul(
                                            acc[h][:],
                                            PT8[m][:, 2 * jb:2 * jb + 2, :],
                                            dx8[:, jb, :, h * 512:(h + 1) * 512],
                                            start=False, stop=last, perf_mode=DR)
                                    if last:
                                        out_m(m, acc)

                    warmt = psw2.tile([128, 128], F16, tag="wrm", name="wrm")
                    for pair in range(2):
                        m0, m1 = 2 * pair, 2 * pair + 1
                        a0, a1 = accs[0], accs[1]
                        if pair == 0:
                            for _ in range(24):
                                nc.tensor.transpose(warmt[:], ident16[:],
                                                    ident16[:])
                        t_sweep1(m0, a0)
                        t_sweep1(m1, a1)
                        if pair == 0:
                            for _ in range(10):
                                nc.tensor.transpose(warmt[:], ident16[:],
                                                    ident16[:])
                        sweep2_pair(m0, m1, a0, a1, seq=(pair == 1))

    nc.compile()
    return nc


_NC_CACHE = None


def kernel(inputs, rotation_params, entangle_params):
    global _NC_CACHE
    if _NC_CACHE is None:
        _NC_CACHE = build_nc()
    nc = _NC_CACHE
    x = np.ascontiguousarray(np.asarray(inputs, np.float32))
    wq = np.asarray(rotation_params, np.float32)
    wk = np.asarray(entangle_params, np.float32)
    w = np.ascontiguousarray(wq @ wk.T)
    xt = np.ascontiguousarray(x.T)
    x8_np = x.astype(E4M3)
    dx8_np = (x - x8_np.astype(np.float32)).astype(E4M3)
    in_maps = []
    for c in range(NC):
        xtr = np.ascontiguousarray(np.roll(xt, -c * NL, axis=1))
        x8r = np.ascontiguousarray(np.roll(x8_np, -c * NL, axis=0))
        dx8r = np.ascontiguousarray(np.roll(dx8_np, -c * NL, axis=0))
        in_maps.append({"w_full": w, "xt_rot": xtr, "x8d": x8r, "dx8d": dx8r})
    r = run_bass_kernel_spmd(nc, in_maps, list(range(NC)))
    return np.concatenate([r.results[c]["out_local"] for c in range(NC)], axis=0)
